# revision 71
# baseline (speedup 1.0000x reference)
"""Trainium2 Bass kernel for nn_Attention_4939212391217.

Full multi-head causal attention block (GPT-2 style):
    qkv = x @ w_attn + b_attn ; q,k,v split ; present = stack(k, v)
    scores = (mask(q @ k^T)) / sqrt(d_k) ; attn = softmax(scores)
    out = attn @ v ; a = out @ w_proj + b_proj ; return (a, present)

Sharding: 8 cores = 2 batches x 4 head-groups (4 heads each).  Each core
computes its batch's qkv slice, causal attention for its 4 heads, and a
partial output projection (w_proj row-sharded); the host sums the 4
partials per batch.  All matmuls run in bf16 with fp32 PSUM accumulation.

Device-side layout notes:
  - x is transposed on the host to x^T [NX, S] so every matmul contracts
    over the partition dimension.
  - Q^T/K^T are produced feature-on-partition ([128, S] tiles holding a
    head *pair*: even head on partitions 0-63, odd head on 64-127), which
    makes the d_k=64 score matmuls row-packable on the PE.
  - Scores are computed transposed (s^T[k, q]) so softmax's k-reduction
    rides along the attn@v matmul via an appended ones column on V
    (psum row 64 = denominator).  exp() runs on the scalar engine with
    the 1/8 scale folded in; causal masking multiplies a 128x128
    triangle into the single diagonal k-tile of each exp tile.
  - Softmax normalization: each accumulator bank is normalized as soon
    as its last (diagonal) k-tile lands; the reciprocal row is broadcast
    across 64 partitions with the GPSIMD partition_broadcast custom
    instruction (the otherwise-idle Pool engine).
"""

import os
import sys
import subprocess
import tempfile

import numpy as np

B, S, NX = 2, 2048, 1024
EXPP_BUFS = int(os.environ.get("EXPP_BUFS", "24"))
MMP_BUFS = int(os.environ.get("MMP_BUFS", "2"))
AUGP_BUFS = int(os.environ.get("AUGP_BUFS", "2"))
CB_BUFS = int(os.environ.get("CB_BUFS", "12"))
N_HEAD = 16
DK = 64
NCORES = 8
HPC = 4            # heads per core
HG = NCORES // B   # head groups (cores per batch)
P = 128
NKT = S // P       # 16 token tiles
CH = 1024          # q chunk size
NCH = S // CH      # 2 chunks


def _build_module():
    import concourse.bacc as bacc
    import concourse.tile as tile
    import concourse.mybir as mybir
    from contextlib import ExitStack

    bf16 = mybir.dt.bfloat16
    f32 = mybir.dt.float32
    Exp = mybir.ActivationFunctionType.Exp

    nc = bacc.Bacc("TRN2", target_bir_lowering=False, debug=False,
                   num_devices=NCORES)

    xT = nc.dram_tensor("xT", [NX, S], bf16, kind="ExternalInput").ap()
    wqk = nc.dram_tensor("wqk", [NX, 4 * P], bf16, kind="ExternalInput").ap()
    wv = nc.dram_tensor("wv", [NX, HPC * DK], bf16, kind="ExternalInput").ap()
    wp = nc.dram_tensor("wp", [2 * P, NX], bf16, kind="ExternalInput").ap()
    bqk = nc.dram_tensor("bqk", [P, 4], f32, kind="ExternalInput").ap()
    bv = nc.dram_tensor("bv", [HPC * DK], f32, kind="ExternalInput").ap()
    tri = nc.dram_tensor("tri", [P, P], bf16, kind="ExternalInput").ap()
    aT = nc.dram_tensor("aT", [NX, S], bf16, kind="ExternalOutput").ap()
    kT_out = nc.dram_tensor("kT_out", [HPC * DK, S], bf16, kind="ExternalOutput").ap()
    v_out = nc.dram_tensor("v_out", [S, HPC * DK], bf16, kind="ExternalOutput").ap()

    with tile.TileContext(nc) as tc:
        with ExitStack() as ctx:
            consts = ctx.enter_context(tc.tile_pool(name="consts", bufs=1))
            big = ctx.enter_context(tc.tile_pool(name="big", bufs=1))
            cb = ctx.enter_context(tc.tile_pool(name="cb", bufs=CB_BUFS))
            expp = ctx.enter_context(tc.tile_pool(name="expp", bufs=EXPP_BUFS))
            mmp = ctx.enter_context(tc.tile_pool(name="mmp", bufs=MMP_BUFS, space="PSUM"))
            ppp = ctx.enter_context(tc.tile_pool(name="ppp", bufs=int(os.environ.get("PPP_BUFS", "2")), space="PSUM"))
            augp = ctx.enter_context(tc.tile_pool(name="augp", bufs=AUGP_BUFS, space="PSUM"))

            # ---- loads ----
            # weights/consts go first on the SP queue; x^T tiles stream in
            # parallel on the Activation HWDGE queue (ACT is idle early on)
            # one DMA per weight tensor (per-DMA issue cost on the
            # sequencer is ~1us; 16 separate loads would pace startup)
            wqk_all = big.tile([P, 8, 4 * P], bf16, tag="wqk", name="wqk_all")
            nc.sync.dma_start(
                wqk_all[:],
                wqk.rearrange("(kt p) m -> p kt m", p=P))
            wqk_sb = [wqk_all[:, kt, :] for kt in range(8)]
            wv_all = big.tile([P, 8, HPC * DK], bf16, tag="wv", name="wv_all")
            nc.sync.dma_start(
                wv_all[:],
                wv.rearrange("(kt p) m -> p kt m", p=P))
            wv_sb = [wv_all[:, kt, :] for kt in range(8)]
            wp_sb = []
            for L in range(2):
                t = big.tile([P, NX], bf16, tag=f"wp{L}")
                nc.sync.dma_start(t[:], wp[L * P:(L + 1) * P, :])
                wp_sb.append(t)
            bqk_sb = consts.tile([P, 4], f32)
            nc.sync.dma_start(bqk_sb[:], bqk)
            bv_sb = consts.tile([P, HPC * DK], f32)
            nc.sync.dma_start(bv_sb[:], bv.partition_broadcast(P))
            tri_sb = consts.tile([P, P], bf16)
            nc.sync.dma_start(tri_sb[:], tri)
            # warm the PE clock gate during the input-DMA wait: ~3us of
            # tiny dummy matmuls so the first real projections run at the
            # full 2.4GHz instead of the cold 1.2GHz
            warm_sb = consts.tile([P, 64], bf16)
            nc.vector.memset(warm_sb[:], 0.0)
            warm_ps = ppp.tile([P, 512], f32, tag="pp", name="warm_ps")
            for _ in range(40):
                nc.tensor.matmul(warm_ps[0:64, 0:64], warm_sb[:], warm_sb[:],
                                 start=True, stop=True)
            # x^T delivered by 512-token COLUMN chunks (one 3D-AP DMA
            # each): chunk j supplies everything attention chunk j needs
            xT_all = big.tile([P, 8, S], bf16, tag="xT", name="xT_all")
            xT3 = xT.rearrange("(kt p) m -> p kt m", p=P)
            for j in range(4):
                nc.scalar.dma_start(
                    xT_all[:, :, 512 * j:512 * (j + 1)],
                    xT3[:, :, 512 * j:512 * (j + 1)])
            xT_sb = [xT_all[:, kt, :] for kt in range(8)]

            # ---- persistent intermediates ----
            # qkT tiles: 0=Q(h0|h1) 1=Q(h2|h3) 2=K(h0|h1) 3=K(h2|h3)
            qkT_sb = [big.tile([P, S], bf16, tag=f"qkT{ft}", name=f"qkT{ft}")
                      for ft in range(4)]
            # v tiles: [128 tokens, 4 heads x (64 V + 1 ones)]
            vaug_sb = [big.tile([P, HPC * (DK + 1)], bf16, tag=f"vaug{tt}",
                                name=f"vaug{tt}") for tt in range(NKT)]
            # attention outputs packed per head pair, proj-ready
            pair_sb = [big.tile([P, S], bf16, tag=f"pair{L}", name=f"pair{L}")
                       for L in range(2)]

            def mm512(ps, lhsT, rhs_tile, col0, width, start, stop):
                """matmul split at 512-col PSUM bank boundaries.

                Writes ps[:, col0:col0+width] = lhsT.T @ rhs_tile, where
                rhs_tile supplies the same column window.  start/stop apply
                to every slice (each column region has its own per-element
                accumulation group).
                """
                c = col0
                while c < col0 + width:
                    e = min((c // 512 + 1) * 512, col0 + width)
                    nc.tensor.matmul(
                        ps[:, c:e], lhsT, rhs_tile[:, c - col0:e - col0],
                        start=start, stop=stop)
                    c = e

            # ---- V projection (normal layout, token-on-partition) ----
            def v_proj(tt):
                ps = ppp.tile([P, 512], f32, tag="pp", name="ps_v")
                for kt in range(8):
                    nc.tensor.matmul(
                        ps[:, :HPC * DK], xT_sb[kt][:, tt * P:(tt + 1) * P],
                        wv_sb[kt][:], start=(kt == 0), stop=(kt == 7))
                nc.vector.memset(vaug_sb[tt][:], 1.0)
                vv = vaug_sb[tt][:].rearrange("p (h j) -> p h j", h=HPC)[:, :, 0:DK]
                with nc.allow_low_precision("bf16 v tiles"):
                    nc.vector.tensor_add(
                        vv, ps[:, :HPC * DK].rearrange("p (h j) -> p h j", h=HPC),
                        bv_sb[:].rearrange("p (h j) -> p h j", h=HPC))
                nc.sync.dma_start(v_out[tt * P:(tt + 1) * P, :], vv)

            # ---- Q^T / K^T projections (feature-on-partition) ----
            def qk_proj(ft, c2, use_ps=False):
                for half in range(2):
                    lo = c2 * CH + half * 512
                    if use_ps:
                        ps = mmp.tile([P, 512], f32, tag="ps", name="ps_qk")
                    else:
                        ps = ppp.tile([P, 512], f32, tag="pp", name="ps_qk")
                    for kt in range(8):
                        nc.tensor.matmul(
                            ps[:], wqk_sb[kt][:, ft * P:(ft + 1) * P],
                            xT_sb[kt][:, lo:lo + 512],
                            start=(kt == 0), stop=(kt == 7))
                    with nc.allow_low_precision("bf16 qkT tiles"):
                        nc.vector.tensor_scalar_add(
                            qkT_sb[ft][:, lo:lo + 512], ps[:],
                            bqk_sb[:, ft:ft + 1])

            # ---- attention for a head pair, one 512-col q-chunk ----
            # Both heads' K=64 score matmuls are emitted back-to-back into
            # one shared [128,1024] PSUM tile (head0 -> bank A, head1 ->
            # bank B, disjoint row groups so hardware runs them
            # concurrently), and a single strided [128,2,w] exp covers both
            # heads -- 80 ACTIVATE ops instead of 96, no garbage columns.
            def attention_pair(hp, c):
                ftq, ftk = hp, 2 + hp
                last_kt = 4 * c + 3
                augs = [augp.tile([DK + 1, 512], f32, tag="aug",
                                  name=f"aug{e}") for e in range(2)]
                for kt in range(last_kt + 1):
                    qs = max(P * kt, 512 * c)
                    w = 512 * (c + 1) - qs
                    col0 = qs - 512 * c
                    sc = mmp.tile([P, CH], f32, tag="ps", name="sc")
                    for e in range(2):
                        nc.tensor.matmul(
                            sc[:, 512 * e:512 * e + w],
                            qkT_sb[ftk][DK * e:DK * e + DK,
                                        kt * P:(kt + 1) * P],
                            qkT_sb[ftq][DK * e:DK * e + DK, qs:qs + w],
                            start=True, stop=True)
                    ex = expp.tile([P, CH], bf16, name="ex")
                    sc3 = sc[:].rearrange("p (e q) -> p e q", e=2)[:, :, :w]
                    ex3 = ex[:].rearrange("p (e q) -> p e q", e=2)[:, :, :w]
                    nc.scalar.activation(ex3, sc3, Exp, scale=0.125)
                    if kt >= 4 * c:
                        exm = ex[:].rearrange("p (e q) -> p e q", e=2)[:, :, 0:P]
                        nc.vector.tensor_mul(
                            exm, exm,
                            tri_sb[:].unsqueeze(1).broadcast_to([P, 2, P]))
                    for e in range(2):
                        h = 2 * hp + e
                        vh = vaug_sb[kt][:, h * (DK + 1):(h + 1) * (DK + 1)]
                        nc.tensor.matmul(
                            augs[e][:, col0:512], vh,
                            ex[:, 512 * e:512 * e + w],
                            start=(kt == 0), stop=(kt == last_kt))
                for e in range(2):
                    pb = DK * e
                    # single cheap copy releases the aug PSUM slot; the
                    # reciprocal/broadcast/normalize chain runs off-slot
                    acp = cb.tile([DK + 1, 512], f32, tag="acp", bufs=4)
                    nc.any.tensor_copy(acp[:], augs[e][:])
                    rcp = cb.tile([P, 512], f32, tag="rcp")
                    nc.vector.reciprocal(rcp[0:1, :], acp[DK:DK + 1, :])
                    rcpb = cb.tile([DK, 512], f32, tag="rcpb")
                    nc.gpsimd.partition_broadcast(rcpb[:], rcp[0:1, :])
                    with nc.allow_low_precision("bf16 attn out"):
                        nc.vector.tensor_mul(
                            pair_sb[hp][pb:pb + DK, 512 * c:512 * (c + 1)],
                            acp[0:DK, :], rcpb[:])

            def out_proj(c):
                # L-outer: both halves reuse the same wp weight load
                for mt in range(8):
                    pss = [ppp.tile([P, 512], f32, tag="pp", name="ps_o")
                           for _ in range(2)]
                    for L in range(2):
                        for half in range(2):
                            lo = c * CH + half * 512
                            nc.tensor.matmul(
                                pss[half][:], wp_sb[L][:, mt * P:(mt + 1) * P],
                                pair_sb[L][:, lo:lo + 512],
                                start=(L == 0), stop=(L == 1))
                    for half in range(2):
                        lo = c * CH + half * 512
                        ob = cb.tile([P, 512], bf16, tag="ob")
                        nc.vector.tensor_copy(ob[:], pss[half][:])
                        nc.sync.dma_start(
                            aT[mt * P:(mt + 1) * P, lo:lo + 512], ob[:])

            # chunk-major emission, minimal-prerequisite-first so the
            # scalar engine's exp stream starts as early as possible
            qk_proj(0, 0)
            qk_proj(2, 0, use_ps=True)
            for tt in range(8):
                v_proj(tt)
            attention_pair(0, 0)
            attention_pair(0, 1)
            qk_proj(1, 0)
            qk_proj(3, 0)
            qk_proj(0, 1)
            qk_proj(2, 1)
            for tt in range(8, NKT):
                v_proj(tt)
            nc.sync.dma_start(kT_out[0:P, :], qkT_sb[2][:])
            attention_pair(1, 0)
            attention_pair(1, 1)
            attention_pair(0, 2)
            qk_proj(1, 1)
            qk_proj(3, 1)
            nc.sync.dma_start(kT_out[P:2 * P, :], qkT_sb[3][:])
            attention_pair(1, 2)
            out_proj(0)
            attention_pair(0, 3)
            attention_pair(1, 3)
            out_proj(1)

    nc.compile()
    return nc


def _make_in_maps(x, w_attn, b_attn):
    import ml_dtypes
    bf = ml_dtypes.bfloat16
    tri = np.triu(np.ones((P, P), dtype=np.float32)).astype(bf)
    xT_b = [np.ascontiguousarray(x[b].T).astype(bf) for b in range(B)]
    in_maps = []
    for cid in range(NCORES):
        b, hg = cid // HG, cid % HG
        qs = slice(hg * 256, (hg + 1) * 256)
        ks = slice(NX + hg * 256, NX + (hg + 1) * 256)
        vs = slice(2 * NX + hg * 256, 2 * NX + (hg + 1) * 256)
        wqk = np.concatenate([w_attn[:, qs], w_attn[:, ks]], axis=1).astype(bf)
        wv = np.ascontiguousarray(w_attn[:, vs]).astype(bf)
        in_maps.append({
            "xT": np.asarray(xT_b[b]),
            "wqk": np.asarray(wqk),
            "wv": np.asarray(wv),
            "wp": None,  # filled below
            "bqk": np.stack([
                b_attn[hg * 256:hg * 256 + 128],
                b_attn[hg * 256 + 128:hg * 256 + 256],
                b_attn[NX + hg * 256:NX + hg * 256 + 128],
                b_attn[NX + hg * 256 + 128:NX + hg * 256 + 256],
            ], axis=1).astype(np.float32),
            "bv": b_attn[2 * NX + hg * 256:2 * NX + (hg + 1) * 256]
                  .astype(np.float32),
            "tri": np.asarray(tri),
        })
    return in_maps


def _kernel_impl(x, w_attn, b_attn, w_proj, b_proj):
    import ml_dtypes
    from concourse import bass_utils
    bf = ml_dtypes.bfloat16

    in_maps = _make_in_maps(x, w_attn, b_attn)
    for cid in range(NCORES):
        hg = cid % HG
        in_maps[cid]["wp"] = np.ascontiguousarray(
            w_proj[hg * 256:(hg + 1) * 256, :]).astype(bf)

    nc = _build_module()
    res = bass_utils.run_bass_kernel_spmd(
        nc, in_maps, core_ids=list(range(NCORES)))

    a = np.zeros((B, S, NX), dtype=np.float32)
    k_full = np.zeros((B, S, NX), dtype=np.float32)
    v_full = np.zeros((B, S, NX), dtype=np.float32)
    for cid in range(NCORES):
        b, hg = cid // HG, cid % HG
        r = res.results[cid]
        a[b] += np.asarray(r["aT"], dtype=bf).astype(np.float32).T
        k_full[b][:, hg * 256:(hg + 1) * 256] = \
            np.asarray(r["kT_out"], dtype=bf).astype(np.float32).T
        v_full[b][:, hg * 256:(hg + 1) * 256] = \
            np.asarray(r["v_out"], dtype=bf).astype(np.float32)
    a += b_proj.astype(np.float32)
    present = np.stack([k_full, v_full])
    return a, present


def kernel(x, w_attn, b_attn, w_proj, b_proj):
    """Run in a subprocess so the harness's JAX config (e.g.
    JAX_PLATFORMS=cpu) can't hide the axon neuron devices."""
    with tempfile.TemporaryDirectory() as td:
        fin = os.path.join(td, "in.npz")
        fout = os.path.join(td, "out.npz")
        np.savez(fin, x=np.asarray(x), w_attn=np.asarray(w_attn),
                 b_attn=np.asarray(b_attn), w_proj=np.asarray(w_proj),
                 b_proj=np.asarray(b_proj))
        env = dict(os.environ)
        env.pop("JAX_PLATFORMS", None)
        subprocess.run(
            [sys.executable, os.path.abspath(__file__), "--child", fin, fout],
            check=True, env=env)
        with np.load(fout) as d:
            return d["a"][...], d["present"][...]


if __name__ == "__main__":
    if len(sys.argv) == 4 and sys.argv[1] == "--child":
        d = np.load(sys.argv[2])
        a, present = _kernel_impl(d["x"], d["w_attn"], d["b_attn"],
                                  d["w_proj"], d["b_proj"])
        np.savez(sys.argv[3], a=a, present=present)



# revision 72
# speedup vs baseline: 1.0020x; 1.0020x over previous
"""Trainium2 Bass kernel for nn_Attention_4939212391217.

Full multi-head causal attention block (GPT-2 style):
    qkv = x @ w_attn + b_attn ; q,k,v split ; present = stack(k, v)
    scores = (mask(q @ k^T)) / sqrt(d_k) ; attn = softmax(scores)
    out = attn @ v ; a = out @ w_proj + b_proj ; return (a, present)

Sharding: 8 cores = 2 batches x 4 head-groups (4 heads each).  Each core
computes its batch's qkv slice, causal attention for its 4 heads, and a
partial output projection (w_proj row-sharded); the host sums the 4
partials per batch.  All matmuls run in bf16 with fp32 PSUM accumulation.

Device-side layout notes:
  - x is transposed on the host to x^T [NX, S] so every matmul contracts
    over the partition dimension.
  - Q^T/K^T are produced feature-on-partition ([128, S] tiles holding a
    head *pair*: even head on partitions 0-63, odd head on 64-127), which
    makes the d_k=64 score matmuls row-packable on the PE.
  - Scores are computed transposed (s^T[k, q]) so softmax's k-reduction
    rides along the attn@v matmul via an appended ones column on V
    (psum row 64 = denominator).  exp() runs on the scalar engine with
    the 1/8 scale folded in; causal masking multiplies a 128x128
    triangle into the single diagonal k-tile of each exp tile.
  - Softmax normalization: each accumulator bank is normalized as soon
    as its last (diagonal) k-tile lands; the reciprocal row is broadcast
    across 64 partitions with the GPSIMD partition_broadcast custom
    instruction (the otherwise-idle Pool engine).
"""

import os
import sys
import subprocess
import tempfile

import numpy as np

B, S, NX = 2, 2048, 1024
EXPP_BUFS = int(os.environ.get("EXPP_BUFS", "28"))
MMP_BUFS = int(os.environ.get("MMP_BUFS", "2"))
AUGP_BUFS = int(os.environ.get("AUGP_BUFS", "2"))
CB_BUFS = int(os.environ.get("CB_BUFS", "12"))
N_HEAD = 16
DK = 64
NCORES = 8
HPC = 4            # heads per core
HG = NCORES // B   # head groups (cores per batch)
P = 128
NKT = S // P       # 16 token tiles
CH = 1024          # q chunk size
NCH = S // CH      # 2 chunks


def _build_module():
    import concourse.bacc as bacc
    import concourse.tile as tile
    import concourse.mybir as mybir
    from contextlib import ExitStack

    bf16 = mybir.dt.bfloat16
    f32 = mybir.dt.float32
    Exp = mybir.ActivationFunctionType.Exp

    nc = bacc.Bacc("TRN2", target_bir_lowering=False, debug=False,
                   num_devices=NCORES)

    xT = nc.dram_tensor("xT", [NX, S], bf16, kind="ExternalInput").ap()
    wqk = nc.dram_tensor("wqk", [NX, 4 * P], bf16, kind="ExternalInput").ap()
    wv = nc.dram_tensor("wv", [NX, HPC * DK], bf16, kind="ExternalInput").ap()
    wp = nc.dram_tensor("wp", [2 * P, NX], bf16, kind="ExternalInput").ap()
    bqk = nc.dram_tensor("bqk", [P, 4], f32, kind="ExternalInput").ap()
    bv = nc.dram_tensor("bv", [HPC * DK], f32, kind="ExternalInput").ap()
    tri = nc.dram_tensor("tri", [P, P], bf16, kind="ExternalInput").ap()
    aT = nc.dram_tensor("aT", [NX, S], bf16, kind="ExternalOutput").ap()
    kT_out = nc.dram_tensor("kT_out", [HPC * DK, S], bf16, kind="ExternalOutput").ap()
    v_out = nc.dram_tensor("v_out", [S, HPC * DK], bf16, kind="ExternalOutput").ap()

    with tile.TileContext(nc) as tc:
        with ExitStack() as ctx:
            consts = ctx.enter_context(tc.tile_pool(name="consts", bufs=1))
            big = ctx.enter_context(tc.tile_pool(name="big", bufs=1))
            cb = ctx.enter_context(tc.tile_pool(name="cb", bufs=CB_BUFS))
            expp = ctx.enter_context(tc.tile_pool(name="expp", bufs=EXPP_BUFS))
            mmp = ctx.enter_context(tc.tile_pool(name="mmp", bufs=MMP_BUFS, space="PSUM"))
            ppp = ctx.enter_context(tc.tile_pool(name="ppp", bufs=int(os.environ.get("PPP_BUFS", "2")), space="PSUM"))
            augp = ctx.enter_context(tc.tile_pool(name="augp", bufs=AUGP_BUFS, space="PSUM"))

            # ---- loads ----
            # weights/consts go first on the SP queue; x^T tiles stream in
            # parallel on the Activation HWDGE queue (ACT is idle early on)
            # one DMA per weight tensor (per-DMA issue cost on the
            # sequencer is ~1us; 16 separate loads would pace startup)
            wqk_all = big.tile([P, 8, 4 * P], bf16, tag="wqk", name="wqk_all")
            nc.sync.dma_start(
                wqk_all[:],
                wqk.rearrange("(kt p) m -> p kt m", p=P))
            wqk_sb = [wqk_all[:, kt, :] for kt in range(8)]
            wv_all = big.tile([P, 8, HPC * DK], bf16, tag="wv", name="wv_all")
            nc.sync.dma_start(
                wv_all[:],
                wv.rearrange("(kt p) m -> p kt m", p=P))
            wv_sb = [wv_all[:, kt, :] for kt in range(8)]
            wp_sb = []
            for L in range(2):
                t = big.tile([P, NX], bf16, tag=f"wp{L}")
                nc.sync.dma_start(t[:], wp[L * P:(L + 1) * P, :])
                wp_sb.append(t)
            bqk_sb = consts.tile([P, 4], f32)
            nc.sync.dma_start(bqk_sb[:], bqk)
            bv_sb = consts.tile([P, HPC * DK], f32)
            nc.sync.dma_start(bv_sb[:], bv.partition_broadcast(P))
            tri_sb = consts.tile([P, P], bf16)
            nc.sync.dma_start(tri_sb[:], tri)
            # warm the PE clock gate during the input-DMA wait: ~3us of
            # tiny dummy matmuls so the first real projections run at the
            # full 2.4GHz instead of the cold 1.2GHz
            warm_sb = consts.tile([P, 64], bf16)
            nc.vector.memset(warm_sb[:], 0.0)
            warm_ps = ppp.tile([P, 512], f32, tag="pp", name="warm_ps")
            for _ in range(40):
                nc.tensor.matmul(warm_ps[0:64, 0:64], warm_sb[:], warm_sb[:],
                                 start=True, stop=True)
            # x^T delivered by 512-token COLUMN chunks (one 3D-AP DMA
            # each): chunk j supplies everything attention chunk j needs
            xT_all = big.tile([P, 8, S], bf16, tag="xT", name="xT_all")
            xT3 = xT.rearrange("(kt p) m -> p kt m", p=P)
            for j in range(4):
                nc.scalar.dma_start(
                    xT_all[:, :, 512 * j:512 * (j + 1)],
                    xT3[:, :, 512 * j:512 * (j + 1)])
            xT_sb = [xT_all[:, kt, :] for kt in range(8)]

            # ---- persistent intermediates ----
            # qkT tiles: 0=Q(h0|h1) 1=Q(h2|h3) 2=K(h0|h1) 3=K(h2|h3)
            qkT_sb = [big.tile([P, S], bf16, tag=f"qkT{ft}", name=f"qkT{ft}")
                      for ft in range(4)]
            # v tiles: [128 tokens, 4 heads x (64 V + 1 ones)]
            vaug_sb = [big.tile([P, HPC * (DK + 1)], bf16, tag=f"vaug{tt}",
                                name=f"vaug{tt}") for tt in range(NKT)]
            # attention outputs packed per head pair, proj-ready
            pair_sb = [big.tile([P, S], bf16, tag=f"pair{L}", name=f"pair{L}")
                       for L in range(2)]

            def mm512(ps, lhsT, rhs_tile, col0, width, start, stop):
                """matmul split at 512-col PSUM bank boundaries.

                Writes ps[:, col0:col0+width] = lhsT.T @ rhs_tile, where
                rhs_tile supplies the same column window.  start/stop apply
                to every slice (each column region has its own per-element
                accumulation group).
                """
                c = col0
                while c < col0 + width:
                    e = min((c // 512 + 1) * 512, col0 + width)
                    nc.tensor.matmul(
                        ps[:, c:e], lhsT, rhs_tile[:, c - col0:e - col0],
                        start=start, stop=stop)
                    c = e

            # ---- V projection (normal layout, token-on-partition) ----
            def v_proj(tt):
                ps = ppp.tile([P, 512], f32, tag="pp", name="ps_v")
                for kt in range(8):
                    nc.tensor.matmul(
                        ps[:, :HPC * DK], xT_sb[kt][:, tt * P:(tt + 1) * P],
                        wv_sb[kt][:], start=(kt == 0), stop=(kt == 7))
                nc.vector.memset(vaug_sb[tt][:], 1.0)
                vv = vaug_sb[tt][:].rearrange("p (h j) -> p h j", h=HPC)[:, :, 0:DK]
                with nc.allow_low_precision("bf16 v tiles"):
                    nc.vector.tensor_add(
                        vv, ps[:, :HPC * DK].rearrange("p (h j) -> p h j", h=HPC),
                        bv_sb[:].rearrange("p (h j) -> p h j", h=HPC))
                nc.sync.dma_start(v_out[tt * P:(tt + 1) * P, :], vv)

            # ---- Q^T / K^T projections (feature-on-partition) ----
            def qk_proj(ft, c2, use_ps=False):
                for half in range(2):
                    lo = c2 * CH + half * 512
                    if use_ps:
                        ps = mmp.tile([P, 512], f32, tag="ps", name="ps_qk")
                    else:
                        ps = ppp.tile([P, 512], f32, tag="pp", name="ps_qk")
                    for kt in range(8):
                        nc.tensor.matmul(
                            ps[:], wqk_sb[kt][:, ft * P:(ft + 1) * P],
                            xT_sb[kt][:, lo:lo + 512],
                            start=(kt == 0), stop=(kt == 7))
                    with nc.allow_low_precision("bf16 qkT tiles"):
                        nc.vector.tensor_scalar_add(
                            qkT_sb[ft][:, lo:lo + 512], ps[:],
                            bqk_sb[:, ft:ft + 1])

            # ---- attention for a head pair, one 512-col q-chunk ----
            # Both heads' K=64 score matmuls are emitted back-to-back into
            # one shared [128,1024] PSUM tile (head0 -> bank A, head1 ->
            # bank B, disjoint row groups so hardware runs them
            # concurrently), and a single strided [128,2,w] exp covers both
            # heads -- 80 ACTIVATE ops instead of 96, no garbage columns.
            def attention_pair(hp, c):
                ftq, ftk = hp, 2 + hp
                last_kt = 4 * c + 3
                augs = [augp.tile([DK + 1, 512], f32, tag="aug",
                                  name=f"aug{e}") for e in range(2)]
                for kt in range(last_kt + 1):
                    qs = max(P * kt, 512 * c)
                    w = 512 * (c + 1) - qs
                    col0 = qs - 512 * c
                    sc = mmp.tile([P, CH], f32, tag="ps", name="sc")
                    for e in range(2):
                        nc.tensor.matmul(
                            sc[:, 512 * e:512 * e + w],
                            qkT_sb[ftk][DK * e:DK * e + DK,
                                        kt * P:(kt + 1) * P],
                            qkT_sb[ftq][DK * e:DK * e + DK, qs:qs + w],
                            start=True, stop=True)
                    ex = expp.tile([P, CH], bf16, name="ex")
                    sc3 = sc[:].rearrange("p (e q) -> p e q", e=2)[:, :, :w]
                    ex3 = ex[:].rearrange("p (e q) -> p e q", e=2)[:, :, :w]
                    nc.scalar.activation(ex3, sc3, Exp, scale=0.125)
                    if kt >= 4 * c:
                        exm = ex[:].rearrange("p (e q) -> p e q", e=2)[:, :, 0:P]
                        nc.vector.tensor_mul(
                            exm, exm,
                            tri_sb[:].unsqueeze(1).broadcast_to([P, 2, P]))
                    for e in range(2):
                        h = 2 * hp + e
                        vh = vaug_sb[kt][:, h * (DK + 1):(h + 1) * (DK + 1)]
                        nc.tensor.matmul(
                            augs[e][:, col0:512], vh,
                            ex[:, 512 * e:512 * e + w],
                            start=(kt == 0), stop=(kt == last_kt))
                for e in range(2):
                    pb = DK * e
                    # single cheap copy releases the aug PSUM slot; the
                    # reciprocal/broadcast/normalize chain runs off-slot
                    acp = cb.tile([DK + 1, 512], f32, tag="acp", bufs=4)
                    nc.any.tensor_copy(acp[:], augs[e][:])
                    rcp = cb.tile([P, 512], f32, tag="rcp")
                    nc.vector.reciprocal(rcp[0:1, :], acp[DK:DK + 1, :])
                    rcpb = cb.tile([DK, 512], f32, tag="rcpb")
                    nc.gpsimd.partition_broadcast(rcpb[:], rcp[0:1, :])
                    with nc.allow_low_precision("bf16 attn out"):
                        nc.vector.tensor_mul(
                            pair_sb[hp][pb:pb + DK, 512 * c:512 * (c + 1)],
                            acp[0:DK, :], rcpb[:])

            def out_proj(c):
                # L-outer: both halves reuse the same wp weight load
                for mt in range(8):
                    pss = [ppp.tile([P, 512], f32, tag="pp", name="ps_o")
                           for _ in range(2)]
                    for L in range(2):
                        for half in range(2):
                            lo = c * CH + half * 512
                            nc.tensor.matmul(
                                pss[half][:], wp_sb[L][:, mt * P:(mt + 1) * P],
                                pair_sb[L][:, lo:lo + 512],
                                start=(L == 0), stop=(L == 1))
                    for half in range(2):
                        lo = c * CH + half * 512
                        ob = cb.tile([P, 512], bf16, tag="ob")
                        nc.vector.tensor_copy(ob[:], pss[half][:])
                        nc.sync.dma_start(
                            aT[mt * P:(mt + 1) * P, lo:lo + 512], ob[:])

            # chunk-major emission, minimal-prerequisite-first so the
            # scalar engine's exp stream starts as early as possible
            qk_proj(0, 0)
            qk_proj(2, 0, use_ps=True)
            for tt in range(8):
                v_proj(tt)
            attention_pair(0, 0)
            attention_pair(0, 1)
            qk_proj(1, 0)
            qk_proj(3, 0)
            qk_proj(0, 1)
            qk_proj(2, 1)
            for tt in range(8, NKT):
                v_proj(tt)
            nc.sync.dma_start(kT_out[0:P, :], qkT_sb[2][:])
            attention_pair(1, 0)
            attention_pair(1, 1)
            attention_pair(0, 2)
            qk_proj(1, 1)
            qk_proj(3, 1)
            nc.sync.dma_start(kT_out[P:2 * P, :], qkT_sb[3][:])
            attention_pair(1, 2)
            out_proj(0)
            attention_pair(0, 3)
            attention_pair(1, 3)
            out_proj(1)

    nc.compile()
    return nc


def _make_in_maps(x, w_attn, b_attn):
    import ml_dtypes
    bf = ml_dtypes.bfloat16
    tri = np.triu(np.ones((P, P), dtype=np.float32)).astype(bf)
    xT_b = [np.ascontiguousarray(x[b].T).astype(bf) for b in range(B)]
    in_maps = []
    for cid in range(NCORES):
        b, hg = cid // HG, cid % HG
        qs = slice(hg * 256, (hg + 1) * 256)
        ks = slice(NX + hg * 256, NX + (hg + 1) * 256)
        vs = slice(2 * NX + hg * 256, 2 * NX + (hg + 1) * 256)
        wqk = np.concatenate([w_attn[:, qs], w_attn[:, ks]], axis=1).astype(bf)
        wv = np.ascontiguousarray(w_attn[:, vs]).astype(bf)
        in_maps.append({
            "xT": np.asarray(xT_b[b]),
            "wqk": np.asarray(wqk),
            "wv": np.asarray(wv),
            "wp": None,  # filled below
            "bqk": np.stack([
                b_attn[hg * 256:hg * 256 + 128],
                b_attn[hg * 256 + 128:hg * 256 + 256],
                b_attn[NX + hg * 256:NX + hg * 256 + 128],
                b_attn[NX + hg * 256 + 128:NX + hg * 256 + 256],
            ], axis=1).astype(np.float32),
            "bv": b_attn[2 * NX + hg * 256:2 * NX + (hg + 1) * 256]
                  .astype(np.float32),
            "tri": np.asarray(tri),
        })
    return in_maps


def _kernel_impl(x, w_attn, b_attn, w_proj, b_proj):
    import ml_dtypes
    from concourse import bass_utils
    bf = ml_dtypes.bfloat16

    in_maps = _make_in_maps(x, w_attn, b_attn)
    for cid in range(NCORES):
        hg = cid % HG
        in_maps[cid]["wp"] = np.ascontiguousarray(
            w_proj[hg * 256:(hg + 1) * 256, :]).astype(bf)

    nc = _build_module()
    res = bass_utils.run_bass_kernel_spmd(
        nc, in_maps, core_ids=list(range(NCORES)))

    a = np.zeros((B, S, NX), dtype=np.float32)
    k_full = np.zeros((B, S, NX), dtype=np.float32)
    v_full = np.zeros((B, S, NX), dtype=np.float32)
    for cid in range(NCORES):
        b, hg = cid // HG, cid % HG
        r = res.results[cid]
        a[b] += np.asarray(r["aT"], dtype=bf).astype(np.float32).T
        k_full[b][:, hg * 256:(hg + 1) * 256] = \
            np.asarray(r["kT_out"], dtype=bf).astype(np.float32).T
        v_full[b][:, hg * 256:(hg + 1) * 256] = \
            np.asarray(r["v_out"], dtype=bf).astype(np.float32)
    a += b_proj.astype(np.float32)
    present = np.stack([k_full, v_full])
    return a, present


def kernel(x, w_attn, b_attn, w_proj, b_proj):
    """Run in a subprocess so the harness's JAX config (e.g.
    JAX_PLATFORMS=cpu) can't hide the axon neuron devices."""
    with tempfile.TemporaryDirectory() as td:
        fin = os.path.join(td, "in.npz")
        fout = os.path.join(td, "out.npz")
        np.savez(fin, x=np.asarray(x), w_attn=np.asarray(w_attn),
                 b_attn=np.asarray(b_attn), w_proj=np.asarray(w_proj),
                 b_proj=np.asarray(b_proj))
        env = dict(os.environ)
        env.pop("JAX_PLATFORMS", None)
        subprocess.run(
            [sys.executable, os.path.abspath(__file__), "--child", fin, fout],
            check=True, env=env)
        with np.load(fout) as d:
            return d["a"][...], d["present"][...]


if __name__ == "__main__":
    if len(sys.argv) == 4 and sys.argv[1] == "--child":
        d = np.load(sys.argv[2])
        a, present = _kernel_impl(d["x"], d["w_attn"], d["b_attn"],
                                  d["w_proj"], d["b_proj"])
        np.savez(sys.argv[3], a=a, present=present)



# revision 73
# speedup vs baseline: 1.0065x; 1.0045x over previous
"""Trainium2 Bass kernel for nn_Attention_4939212391217.

Full multi-head causal attention block (GPT-2 style):
    qkv = x @ w_attn + b_attn ; q,k,v split ; present = stack(k, v)
    scores = (mask(q @ k^T)) / sqrt(d_k) ; attn = softmax(scores)
    out = attn @ v ; a = out @ w_proj + b_proj ; return (a, present)

Sharding: 8 cores = 2 batches x 4 head-groups (4 heads each).  Each core
computes its batch's qkv slice, causal attention for its 4 heads, and a
partial output projection (w_proj row-sharded); the host sums the 4
partials per batch.  All matmuls run in bf16 with fp32 PSUM accumulation.

Device-side layout notes:
  - x is transposed on the host to x^T [NX, S] so every matmul contracts
    over the partition dimension.
  - Q^T/K^T are produced feature-on-partition ([128, S] tiles holding a
    head *pair*: even head on partitions 0-63, odd head on 64-127), which
    makes the d_k=64 score matmuls row-packable on the PE.
  - Scores are computed transposed (s^T[k, q]) so softmax's k-reduction
    rides along the attn@v matmul via an appended ones column on V
    (psum row 64 = denominator).  exp() runs on the scalar engine with
    the 1/8 scale folded in; causal masking multiplies a 128x128
    triangle into the single diagonal k-tile of each exp tile.
  - Softmax normalization: each accumulator bank is normalized as soon
    as its last (diagonal) k-tile lands; the reciprocal row is broadcast
    across 64 partitions with the GPSIMD partition_broadcast custom
    instruction (the otherwise-idle Pool engine).
"""

import os
import sys
import subprocess
import tempfile

import numpy as np

B, S, NX = 2, 2048, 1024
EXPP_BUFS = int(os.environ.get("EXPP_BUFS", "38"))
MMP_BUFS = int(os.environ.get("MMP_BUFS", "2"))
AUGP_BUFS = int(os.environ.get("AUGP_BUFS", "2"))
CB_BUFS = int(os.environ.get("CB_BUFS", "7"))
N_HEAD = 16
DK = 64
NCORES = 8
HPC = 4            # heads per core
HG = NCORES // B   # head groups (cores per batch)
P = 128
NKT = S // P       # 16 token tiles
CH = 1024          # q chunk size
NCH = S // CH      # 2 chunks


def _build_module():
    import concourse.bacc as bacc
    import concourse.tile as tile
    import concourse.mybir as mybir
    from contextlib import ExitStack

    bf16 = mybir.dt.bfloat16
    f32 = mybir.dt.float32
    Exp = mybir.ActivationFunctionType.Exp

    nc = bacc.Bacc("TRN2", target_bir_lowering=False, debug=False,
                   num_devices=NCORES)

    xT = nc.dram_tensor("xT", [NX, S], bf16, kind="ExternalInput").ap()
    wqk = nc.dram_tensor("wqk", [NX, 4 * P], bf16, kind="ExternalInput").ap()
    wv = nc.dram_tensor("wv", [NX, HPC * DK], bf16, kind="ExternalInput").ap()
    wp = nc.dram_tensor("wp", [2 * P, NX], bf16, kind="ExternalInput").ap()
    bqk = nc.dram_tensor("bqk", [P, 4], f32, kind="ExternalInput").ap()
    bv = nc.dram_tensor("bv", [HPC * DK], f32, kind="ExternalInput").ap()
    tri = nc.dram_tensor("tri", [P, P], bf16, kind="ExternalInput").ap()
    aT = nc.dram_tensor("aT", [NX, S], bf16, kind="ExternalOutput").ap()
    kT_out = nc.dram_tensor("kT_out", [HPC * DK, S], bf16, kind="ExternalOutput").ap()
    v_out = nc.dram_tensor("v_out", [S, HPC * DK], bf16, kind="ExternalOutput").ap()

    with tile.TileContext(nc) as tc:
        with ExitStack() as ctx:
            consts = ctx.enter_context(tc.tile_pool(name="consts", bufs=1))
            big = ctx.enter_context(tc.tile_pool(name="big", bufs=1))
            cb = ctx.enter_context(tc.tile_pool(name="cb", bufs=CB_BUFS))
            expp = ctx.enter_context(tc.tile_pool(name="expp", bufs=EXPP_BUFS))
            mmp = ctx.enter_context(tc.tile_pool(name="mmp", bufs=MMP_BUFS, space="PSUM"))
            ppp = ctx.enter_context(tc.tile_pool(name="ppp", bufs=int(os.environ.get("PPP_BUFS", "2")), space="PSUM"))
            augp = ctx.enter_context(tc.tile_pool(name="augp", bufs=AUGP_BUFS, space="PSUM"))

            # ---- loads ----
            # weights/consts go first on the SP queue; x^T tiles stream in
            # parallel on the Activation HWDGE queue (ACT is idle early on)
            # one DMA per weight tensor (per-DMA issue cost on the
            # sequencer is ~1us; 16 separate loads would pace startup)
            wqk_all = big.tile([P, 8, 4 * P], bf16, tag="wqk", name="wqk_all")
            nc.sync.dma_start(
                wqk_all[:],
                wqk.rearrange("(kt p) m -> p kt m", p=P))
            wqk_sb = [wqk_all[:, kt, :] for kt in range(8)]
            wv_all = big.tile([P, 8, HPC * DK], bf16, tag="wv", name="wv_all")
            nc.sync.dma_start(
                wv_all[:],
                wv.rearrange("(kt p) m -> p kt m", p=P))
            wv_sb = [wv_all[:, kt, :] for kt in range(8)]
            wp_sb = []
            for L in range(2):
                t = big.tile([P, NX], bf16, tag=f"wp{L}")
                nc.sync.dma_start(t[:], wp[L * P:(L + 1) * P, :])
                wp_sb.append(t)
            bqk_sb = consts.tile([P, 4], f32)
            nc.sync.dma_start(bqk_sb[:], bqk)
            bv_sb = consts.tile([P, HPC * DK], f32)
            nc.sync.dma_start(bv_sb[:], bv.partition_broadcast(P))
            tri_sb = consts.tile([P, P], bf16)
            nc.sync.dma_start(tri_sb[:], tri)
            # warm the PE clock gate during the input-DMA wait: ~3us of
            # tiny dummy matmuls so the first real projections run at the
            # full 2.4GHz instead of the cold 1.2GHz
            warm_sb = consts.tile([P, 64], bf16)
            nc.vector.memset(warm_sb[:], 0.0)
            warm_ps = ppp.tile([P, 512], f32, tag="pp", name="warm_ps")
            for _ in range(40):
                nc.tensor.matmul(warm_ps[0:64, 0:64], warm_sb[:], warm_sb[:],
                                 start=True, stop=True)
            # x^T delivered by 512-token COLUMN chunks (one 3D-AP DMA
            # each): chunk j supplies everything attention chunk j needs
            xT_all = big.tile([P, 8, S], bf16, tag="xT", name="xT_all")
            xT3 = xT.rearrange("(kt p) m -> p kt m", p=P)
            for j in range(4):
                nc.scalar.dma_start(
                    xT_all[:, :, 512 * j:512 * (j + 1)],
                    xT3[:, :, 512 * j:512 * (j + 1)])
            xT_sb = [xT_all[:, kt, :] for kt in range(8)]

            # ---- persistent intermediates ----
            # qkT tiles: 0=Q(h0|h1) 1=Q(h2|h3) 2=K(h0|h1) 3=K(h2|h3)
            qkT_sb = [big.tile([P, S], bf16, tag=f"qkT{ft}", name=f"qkT{ft}")
                      for ft in range(4)]
            # v tiles: [128 tokens, 4 heads x (64 V + 1 ones)]
            vaug_sb = [big.tile([P, HPC * (DK + 1)], bf16, tag=f"vaug{tt}",
                                name=f"vaug{tt}") for tt in range(NKT)]
            # attention outputs packed per head pair, proj-ready
            pair_sb = [big.tile([P, S], bf16, tag=f"pair{L}", name=f"pair{L}")
                       for L in range(2)]

            def mm512(ps, lhsT, rhs_tile, col0, width, start, stop):
                """matmul split at 512-col PSUM bank boundaries.

                Writes ps[:, col0:col0+width] = lhsT.T @ rhs_tile, where
                rhs_tile supplies the same column window.  start/stop apply
                to every slice (each column region has its own per-element
                accumulation group).
                """
                c = col0
                while c < col0 + width:
                    e = min((c // 512 + 1) * 512, col0 + width)
                    nc.tensor.matmul(
                        ps[:, c:e], lhsT, rhs_tile[:, c - col0:e - col0],
                        start=start, stop=stop)
                    c = e

            # ---- V projection (normal layout, token-on-partition) ----
            def v_proj(tt):
                ps = ppp.tile([P, 512], f32, tag="pp", name="ps_v")
                for kt in range(8):
                    nc.tensor.matmul(
                        ps[:, :HPC * DK], xT_sb[kt][:, tt * P:(tt + 1) * P],
                        wv_sb[kt][:], start=(kt == 0), stop=(kt == 7))
                nc.vector.memset(vaug_sb[tt][:], 1.0)
                vv = vaug_sb[tt][:].rearrange("p (h j) -> p h j", h=HPC)[:, :, 0:DK]
                with nc.allow_low_precision("bf16 v tiles"):
                    nc.vector.tensor_add(
                        vv, ps[:, :HPC * DK].rearrange("p (h j) -> p h j", h=HPC),
                        bv_sb[:].rearrange("p (h j) -> p h j", h=HPC))
                nc.sync.dma_start(v_out[tt * P:(tt + 1) * P, :], vv)

            # ---- Q^T / K^T projections (feature-on-partition) ----
            def qk_proj(ft, c2, use_ps=False):
                for half in range(2):
                    lo = c2 * CH + half * 512
                    if use_ps:
                        ps = mmp.tile([P, 512], f32, tag="ps", name="ps_qk")
                    else:
                        ps = ppp.tile([P, 512], f32, tag="pp", name="ps_qk")
                    for kt in range(8):
                        nc.tensor.matmul(
                            ps[:], wqk_sb[kt][:, ft * P:(ft + 1) * P],
                            xT_sb[kt][:, lo:lo + 512],
                            start=(kt == 0), stop=(kt == 7))
                    with nc.allow_low_precision("bf16 qkT tiles"):
                        nc.vector.tensor_scalar_add(
                            qkT_sb[ft][:, lo:lo + 512], ps[:],
                            bqk_sb[:, ft:ft + 1])

            # ---- attention for a head pair, one 512-col q-chunk ----
            # Both heads' K=64 score matmuls are emitted back-to-back into
            # one shared [128,1024] PSUM tile (head0 -> bank A, head1 ->
            # bank B, disjoint row groups so hardware runs them
            # concurrently), and a single strided [128,2,w] exp covers both
            # heads -- 80 ACTIVATE ops instead of 96, no garbage columns.
            def attention_pair(hp, c):
                ftq, ftk = hp, 2 + hp
                last_kt = 4 * c + 3
                augs = [augp.tile([DK + 1, 512], f32, tag="aug",
                                  name=f"aug{e}") for e in range(2)]
                for kt in range(last_kt + 1):
                    qs = max(P * kt, 512 * c)
                    w = 512 * (c + 1) - qs
                    col0 = qs - 512 * c
                    sc = mmp.tile([P, CH], f32, tag="ps", name="sc")
                    for e in range(2):
                        nc.tensor.matmul(
                            sc[:, 512 * e:512 * e + w],
                            qkT_sb[ftk][DK * e:DK * e + DK,
                                        kt * P:(kt + 1) * P],
                            qkT_sb[ftq][DK * e:DK * e + DK, qs:qs + w],
                            start=True, stop=True)
                    ex = expp.tile([P, CH], bf16, name="ex")
                    sc3 = sc[:].rearrange("p (e q) -> p e q", e=2)[:, :, :w]
                    ex3 = ex[:].rearrange("p (e q) -> p e q", e=2)[:, :, :w]
                    nc.scalar.activation(ex3, sc3, Exp, scale=0.125)
                    if kt >= 4 * c:
                        exm = ex[:].rearrange("p (e q) -> p e q", e=2)[:, :, 0:P]
                        nc.vector.tensor_mul(
                            exm, exm,
                            tri_sb[:].unsqueeze(1).broadcast_to([P, 2, P]))
                    for e in range(2):
                        h = 2 * hp + e
                        vh = vaug_sb[kt][:, h * (DK + 1):(h + 1) * (DK + 1)]
                        nc.tensor.matmul(
                            augs[e][:, col0:512], vh,
                            ex[:, 512 * e:512 * e + w],
                            start=(kt == 0), stop=(kt == last_kt))
                for e in range(2):
                    pb = DK * e
                    # single cheap copy releases the aug PSUM slot; the
                    # reciprocal/broadcast/normalize chain runs off-slot
                    acp = cb.tile([DK + 1, 512], f32, tag="acp", bufs=4)
                    nc.any.tensor_copy(acp[:], augs[e][:])
                    rcp = cb.tile([P, 512], f32, tag="rcp")
                    nc.vector.reciprocal(rcp[0:1, :], acp[DK:DK + 1, :])
                    rcpb = cb.tile([DK, 512], f32, tag="rcpb")
                    nc.gpsimd.partition_broadcast(rcpb[:], rcp[0:1, :])
                    with nc.allow_low_precision("bf16 attn out"):
                        nc.vector.tensor_mul(
                            pair_sb[hp][pb:pb + DK, 512 * c:512 * (c + 1)],
                            acp[0:DK, :], rcpb[:])

            def out_proj(c):
                # L-outer: both halves reuse the same wp weight load
                for mt in range(8):
                    pss = [ppp.tile([P, 512], f32, tag="pp", name="ps_o")
                           for _ in range(2)]
                    for L in range(2):
                        for half in range(2):
                            lo = c * CH + half * 512
                            nc.tensor.matmul(
                                pss[half][:], wp_sb[L][:, mt * P:(mt + 1) * P],
                                pair_sb[L][:, lo:lo + 512],
                                start=(L == 0), stop=(L == 1))
                    for half in range(2):
                        lo = c * CH + half * 512
                        ob = cb.tile([P, 512], bf16, tag="ob")
                        nc.vector.tensor_copy(ob[:], pss[half][:])
                        nc.sync.dma_start(
                            aT[mt * P:(mt + 1) * P, lo:lo + 512], ob[:])

            # chunk-major emission, minimal-prerequisite-first so the
            # scalar engine's exp stream starts as early as possible
            qk_proj(0, 0)
            qk_proj(2, 0, use_ps=True)
            for tt in range(8):
                v_proj(tt)
            attention_pair(0, 0)
            attention_pair(0, 1)
            qk_proj(1, 0)
            qk_proj(3, 0)
            qk_proj(0, 1)
            qk_proj(2, 1)
            for tt in range(8, NKT):
                v_proj(tt)
            nc.sync.dma_start(kT_out[0:P, :], qkT_sb[2][:])
            attention_pair(1, 0)
            attention_pair(1, 1)
            attention_pair(0, 2)
            qk_proj(1, 1)
            qk_proj(3, 1)
            nc.sync.dma_start(kT_out[P:2 * P, :], qkT_sb[3][:])
            attention_pair(1, 2)
            out_proj(0)
            attention_pair(0, 3)
            attention_pair(1, 3)
            out_proj(1)

    nc.compile()
    return nc


def _make_in_maps(x, w_attn, b_attn):
    import ml_dtypes
    bf = ml_dtypes.bfloat16
    tri = np.triu(np.ones((P, P), dtype=np.float32)).astype(bf)
    xT_b = [np.ascontiguousarray(x[b].T).astype(bf) for b in range(B)]
    in_maps = []
    for cid in range(NCORES):
        b, hg = cid // HG, cid % HG
        qs = slice(hg * 256, (hg + 1) * 256)
        ks = slice(NX + hg * 256, NX + (hg + 1) * 256)
        vs = slice(2 * NX + hg * 256, 2 * NX + (hg + 1) * 256)
        wqk = np.concatenate([w_attn[:, qs], w_attn[:, ks]], axis=1).astype(bf)
        wv = np.ascontiguousarray(w_attn[:, vs]).astype(bf)
        in_maps.append({
            "xT": np.asarray(xT_b[b]),
            "wqk": np.asarray(wqk),
            "wv": np.asarray(wv),
            "wp": None,  # filled below
            "bqk": np.stack([
                b_attn[hg * 256:hg * 256 + 128],
                b_attn[hg * 256 + 128:hg * 256 + 256],
                b_attn[NX + hg * 256:NX + hg * 256 + 128],
                b_attn[NX + hg * 256 + 128:NX + hg * 256 + 256],
            ], axis=1).astype(np.float32),
            "bv": b_attn[2 * NX + hg * 256:2 * NX + (hg + 1) * 256]
                  .astype(np.float32),
            "tri": np.asarray(tri),
        })
    return in_maps


def _kernel_impl(x, w_attn, b_attn, w_proj, b_proj):
    import ml_dtypes
    from concourse import bass_utils
    bf = ml_dtypes.bfloat16

    in_maps = _make_in_maps(x, w_attn, b_attn)
    for cid in range(NCORES):
        hg = cid % HG
        in_maps[cid]["wp"] = np.ascontiguousarray(
            w_proj[hg * 256:(hg + 1) * 256, :]).astype(bf)

    nc = _build_module()
    res = bass_utils.run_bass_kernel_spmd(
        nc, in_maps, core_ids=list(range(NCORES)))

    a = np.zeros((B, S, NX), dtype=np.float32)
    k_full = np.zeros((B, S, NX), dtype=np.float32)
    v_full = np.zeros((B, S, NX), dtype=np.float32)
    for cid in range(NCORES):
        b, hg = cid // HG, cid % HG
        r = res.results[cid]
        a[b] += np.asarray(r["aT"], dtype=bf).astype(np.float32).T
        k_full[b][:, hg * 256:(hg + 1) * 256] = \
            np.asarray(r["kT_out"], dtype=bf).astype(np.float32).T
        v_full[b][:, hg * 256:(hg + 1) * 256] = \
            np.asarray(r["v_out"], dtype=bf).astype(np.float32)
    a += b_proj.astype(np.float32)
    present = np.stack([k_full, v_full])
    return a, present


def kernel(x, w_attn, b_attn, w_proj, b_proj):
    """Run in a subprocess so the harness's JAX config (e.g.
    JAX_PLATFORMS=cpu) can't hide the axon neuron devices."""
    with tempfile.TemporaryDirectory() as td:
        fin = os.path.join(td, "in.npz")
        fout = os.path.join(td, "out.npz")
        np.savez(fin, x=np.asarray(x), w_attn=np.asarray(w_attn),
                 b_attn=np.asarray(b_attn), w_proj=np.asarray(w_proj),
                 b_proj=np.asarray(b_proj))
        env = dict(os.environ)
        env.pop("JAX_PLATFORMS", None)
        subprocess.run(
            [sys.executable, os.path.abspath(__file__), "--child", fin, fout],
            check=True, env=env)
        with np.load(fout) as d:
            return d["a"][...], d["present"][...]


if __name__ == "__main__":
    if len(sys.argv) == 4 and sys.argv[1] == "--child":
        d = np.load(sys.argv[2])
        a, present = _kernel_impl(d["x"], d["w_attn"], d["b_attn"],
                                  d["w_proj"], d["b_proj"])
        np.savez(sys.argv[3], a=a, present=present)

